# revision 10
# baseline (speedup 1.0000x reference)
"""ATSS assignment kernel for Trainium2 (8 NeuronCores, data-parallel over batch).

Pipeline per core (2 images per core):
  1. One fused PE matmul per 512-chunk computes u = g.p - |p|^2/2 via a
     4-row contraction [gx, gy, -1/2, -1/2] x [px, py, px^2, py^2]
     (ranking-equivalent to negated squared distance).
  2. DVE tensor_reduce takes per-16-chunk maxima straight out of PSUM.
  3. A max8/max_index/match_replace cascade selects the top-72 chunks per
     GT row (any chunk holding a true top-64 pred ranks <= 64 by chunk-max).
  4. Chunk ids are transposed into the gpsimd wrapped-index layout with a
     few small DMAs and ALL 72*128 chunk blocks are fetched by a single
     dma_gather (SWDGE cost ~994ns + 0.34ns/desc instead of 994ns per
     block-column with per-slot indirect DMAs).
  5. Exact fp32 squared distances are recomputed for the 1152 candidates
     and a second cascade extracts the exact top-64 (jax top_k semantics).
  6. A rank map built by local_scatter compacts the original pred indices
     to rank order; a second dma_gather fetches the 64 selected boxes per
     GT row and the IoU / mean+std threshold / center-inside epilogue runs
     entirely in compact [G, 64] rank space.
"""

import sys

import numpy as np

if "/opt/trn_rl_repo" not in sys.path:
    sys.path.insert(0, "/opt/trn_rl_repo")

import concourse.bass as bass
import concourse.mybir as mybir
import concourse.tile as tile
from concourse import bacc

F32 = mybir.dt.float32
U8 = mybir.dt.uint8
U16 = mybir.dt.uint16
I16 = mybir.dt.int16
I32 = mybir.dt.int32
AL = mybir.AluOpType
ACT = mybir.ActivationFunctionType
AX = mybir.AxisListType

B, N, G, K = 16, 16384, 128, 64
NCORES = 8
BPC = B // NCORES          # batches (images) per core
CW = 16                    # chunk width for the prefilter
NCH = N // CW              # 1024 chunks per row
NSEL = 72                  # chunks kept per row (>= worst-case 65 + margin)
NIT1 = NSEL // 8           # cascade-1 iterations
CAND = NSEL * CW           # 1152 candidate preds per row
NIT2 = K // 8              # cascade-2 iterations
MMF = 512                  # matmul free-dim chunk (one PSUM bank)
NEG = -1e30


def _wrap_idx_dmas(nc, idxw, src16, ns):
    """Transpose [128, ns] per-partition ids into the gpsimd wrapped index
    layout idxw[16k+p, s*8+q] = src[q*16+p, s], replicated into all eight
    16-partition groups."""
    dst3 = idxw[0:16, :].rearrange("p (s q) -> p s q", q=8)
    for q in range(8):
        nc.sync.dma_start(dst3[:, :, q], src16[16 * q : 16 * (q + 1), :])
    for npart in (16, 32, 64):
        nc.sync.dma_start(idxw[npart : 2 * npart, :], idxw[0:npart, :])


def build_program(num_devices=NCORES, debug_taps=False, stop_after=99):
    nc = bacc.Bacc(
        "TRN2",
        debug=False,
        target_bir_lowering=False,
        num_devices=num_devices,
    )
    ct_in = nc.dram_tensor("ct_in", [BPC, 2, G + N], F32, kind="ExternalInput")
    pred_cc = nc.dram_tensor("pred_cc", [BPC, NCH, 4 * CW], F32, kind="ExternalInput")
    boxes64 = nc.dram_tensor("boxes64", [BPC, N, 64], F32, kind="ExternalInput")
    gt_boxes = nc.dram_tensor("gt_boxes", [BPC, G, 4], F32, kind="ExternalInput")
    io_e_in = nc.dram_tensor("io_e_in", [G, CAND], U16, kind="ExternalInput")
    io_r_in = nc.dram_tensor("io_r_in", [G, K], I16, kind="ExternalInput")
    out_ious = nc.dram_tensor("out_ious", [BPC, G, K], F32, kind="ExternalOutput")
    out_mask = nc.dram_tensor("out_mask", [BPC, G, K], U8, kind="ExternalOutput")
    out_kidx = nc.dram_tensor("out_kidx", [BPC, G, K], I32, kind="ExternalOutput")

    with tile.TileContext(nc) as tc:
        _emit(nc, tc, ct_in, pred_cc, boxes64, gt_boxes, io_e_in, io_r_in,
              out_ious, out_mask, out_kidx)
    nc.compile()
    return nc


def _emit(nc, tc, ct_in, pred_cc, boxes64, gt_boxes, io_e_in, io_r_in,
          out_ious, out_mask, out_kidx):
    with (
        tc.tile_pool(name="const", bufs=1) as cpool,
        tc.tile_pool(name="sb", bufs=2) as sb,
        tc.tile_pool(name="cw", bufs=1) as cw,
        tc.tile_pool(name="rhsp", bufs=2) as rhsp,
        tc.tile_pool(name="ps0", bufs=2, space="PSUM") as ps0,
        tc.tile_pool(name="ps1", bufs=2, space="PSUM") as ps1,
    ):
        # iota 0..15 repeated per chunk slot (j%16) and ranks 1..64
        io_e = cpool.tile([G, CAND], U16)
        nc.sync.dma_start(io_e, io_e_in.ap())
        io_r = cpool.tile([G, K], I16)
        nc.sync.dma_start(io_r, io_r_in.ap())

        per_b = [dict() for _ in range(BPC)]
        for b in range(BPC):
            # ---------------- GT prep ----------------
            gt_sb = sb.tile([G, 4], F32)
            nc.sync.dma_start(gt_sb, gt_boxes.ap()[b])
            gx = gt_sb[:, 0:1]
            gy = gt_sb[:, 1:2]

            ghw = sb.tile([G, 1], F32, tag="ghw")
            nc.vector.tensor_scalar_mul(ghw, gt_sb[:, 2:3], 0.5)
            ghh = sb.tile([G, 1], F32, tag="ghh")
            nc.vector.tensor_scalar_mul(ghh, gt_sb[:, 3:4], 0.5)
            x1 = sb.tile([G, 1], F32, tag="x1")
            nc.vector.tensor_sub(x1, gx, ghw)
            x2 = sb.tile([G, 1], F32, tag="x2")
            nc.vector.tensor_add(x2, gx, ghw)
            y1 = sb.tile([G, 1], F32, tag="y1")
            nc.vector.tensor_sub(y1, gy, ghh)
            y2 = sb.tile([G, 1], F32, tag="y2")
            nc.vector.tensor_add(y2, gy, ghh)
            aw = sb.tile([G, 1], F32, tag="aw")
            nc.vector.tensor_sub(aw, x2, x1)
            ah = sb.tile([G, 1], F32, tag="ah")
            nc.vector.tensor_sub(ah, y2, y1)
            area_a = sb.tile([G, 1], F32, tag="area_a")
            nc.vector.tensor_mul(area_a, aw, ah)

            # stage row c = pred_c (N); gt centers in a separate tile
            stage = rhsp.tile([2, N], F32)
            nc.sync.dma_start(stage, ct_in.ap()[b][:, G:])
            gtc = sb.tile([2, G], F32, tag="gtc")
            nc.sync.dma_start(gtc, ct_in.ap()[b][:, 0:G])
            negh = sb.tile([2, G], F32, tag="negh")
            nc.vector.memset(negh, -0.5)

            # ---------------- scores + chunk maxima ----------------
            m16 = cw.tile([G, NCH], F32, tag="m16")
            psp = ps0 if b % 2 == 0 else ps1
            for j2 in range(N // (2 * MMF)):
                mm = psp.tile([G, 2 * MMF], F32, tag="mm")
                for h in range(2):
                    j = 2 * j2 + h
                    sq2 = sb.tile([2, MMF], F32, tag="sq2")
                    nc.scalar.activation(
                        sq2, stage[:, j * MMF : (j + 1) * MMF],
                        ACT.Square)
                    nc.tensor.matmul(
                        mm[:, h * MMF : (h + 1) * MMF], lhsT=gtc,
                        rhs=stage[:, j * MMF : (j + 1) * MMF],
                        start=True, stop=False,
                    )
                    nc.tensor.matmul(
                        mm[:, h * MMF : (h + 1) * MMF], lhsT=negh, rhs=sq2,
                        start=False, stop=True,
                    )
                nc.vector.tensor_reduce(
                    m16[:, j2 * (2 * MMF // CW) : (j2 + 1) * (2 * MMF // CW)],
                    mm.rearrange("g (c w) -> g c w", w=CW),
                    axis=AX.X, op=AL.max,
                )

            # ---- cascade 1: top-NSEL chunks per GT row ----
            ci16 = sb.tile([G, NSEL], U16)
            for i in range(NIT1):
                v8 = sb.tile([G, 8], F32, tag="v8")
                nc.vector.max(v8, m16)
                nc.vector.max_index(ci16[:, i * 8 : (i + 1) * 8], v8, m16)
                if i < NIT1 - 1:
                    nc.vector.match_replace(m16, v8, m16, NEG)

            # ---- batched gather of all selected chunk blocks ----
            idxw = cw.tile([128, NSEL * 8], I16, tag="idxw")
            _wrap_idx_dmas(nc, idxw, ci16.bitcast(I16), NSEL)
            cand = cw.tile([G, NSEL, 4 * CW], F32, tag=f"cand{b}")
            nc.gpsimd.dma_gather(
                out_ap=cand, in_ap=pred_cc.ap()[b], idxs_ap=idxw,
                num_idxs=G * NSEL, num_idxs_reg=G * NSEL,
                elem_size=4 * CW, single_packet=False,
            )
            per_b[b] = dict(gt_sb=gt_sb, x1=x1, x2=x2, y1=y1, y2=y2,
                            area_a=area_a, ci16=ci16, cand=cand)

        for b in range(BPC):
            d = per_b[b]
            gt_sb, ci16, cand = d["gt_sb"], d["ci16"], d["cand"]
            gx = gt_sb[:, 0:1]
            gy = gt_sb[:, 1:2]
            cx_c = cand[:, :, 0 * CW : 1 * CW]
            cy_c = cand[:, :, 1 * CW : 2 * CW]

            # ---------------- exact negated d2 ----------------
            T1 = cw.tile([G, CAND], F32, tag="T1")
            T2 = cw.tile([G, CAND], F32, tag="T2")
            uc = cw.tile([G, CAND], F32, tag="T3")

            nc.vector.tensor_scalar_sub(T1, cx_c, gx)        # dx
            nc.scalar.activation(T1, T1, ACT.Square)
            nc.vector.tensor_scalar_sub(T2, cy_c, gy)        # dy
            nc.scalar.activation(T2, T2, ACT.Square)
            nc.vector.scalar_tensor_tensor(
                uc, in0=T1, scalar=-1.0, in1=T2, op0=AL.mult, op1=AL.subtract
            )

            # ---------------- cascade 2: exact top-64 ----------------
            pos16 = sb.tile([G, K], U16)
            for i in range(NIT2):
                v8b = sb.tile([G, 8], F32, tag="v8b")
                nc.vector.max(v8b, uc)
                nc.vector.max_index(pos16[:, i * 8 : (i + 1) * 8], v8b, uc)
                if i < NIT2 - 1:
                    nc.vector.match_replace(uc, v8b, uc, NEG)

            # ---------------- rank map + original indices ----------------
            pos_i = sb.tile([G, K], I16, tag="pos_i")
            nc.vector.tensor_copy(pos_i, pos16)
            rkm = cw.tile([G, CAND], I16, tag="t_rkm")
            nc.gpsimd.local_scatter(rkm, io_r, pos_i, channels=G,
                                    num_elems=CAND, num_idxs=K)
            nc.vector.tensor_scalar(rkm, rkm, 1, None, op0=AL.subtract)

            # cid[g, j] = ci16[g, j//16]*16 + j%16  (global pred index)
            cid = cw.tile([G, CAND], U16, tag="t_cid")
            for e in range(CW):
                nc.scalar.activation(cid[:, e::CW], ci16, ACT.Copy, scale=16.0)
            nc.vector.tensor_tensor(cid, cid, io_e, op=AL.add)

            kidx16 = sb.tile([G, K], U16, tag="kidx16")
            nc.gpsimd.local_scatter(kidx16, cid, rkm, channels=G,
                                    num_elems=K, num_idxs=CAND)
            kidxi = sb.tile([G, K], I32, tag="kidxi")
            nc.vector.tensor_copy(kidxi, kidx16)
            nc.sync.dma_start(out_kidx.ap()[b], kidxi)

            # ---------------- gather the 64 ranked boxes ----------------
            idxw2 = cw.tile([128, K * 8], I16, tag="idxw2")
            _wrap_idx_dmas(nc, idxw2, kidx16.bitcast(I16), K)
            kbfull = cw.tile([G, NSEL, 4 * CW], F32, tag=f"cand{b}")
            kb = kbfull[:, 0:K, :]
            nc.gpsimd.dma_gather(
                out_ap=kb, in_ap=boxes64.ap()[b], idxs_ap=idxw2,
                num_idxs=G * K, num_idxs_reg=G * K,
                elem_size=64, single_packet=False,
            )
            d["kb"] = kb

        for b in range(BPC):
            d = per_b[b]
            x1, x2, y1, y2 = d["x1"], d["x2"], d["y1"], d["y2"]
            area_a, kb = d["area_a"], d["kb"]

            # ---------------- rank-space epilogue [G, K] ----------------
            cxr = sb.tile([G, K], F32, tag="cxr")
            nc.vector.tensor_copy(cxr, kb[:, :, 0:1].rearrange("g k o -> g (k o)"))
            cyr = sb.tile([G, K], F32, tag="cyr")
            nc.vector.tensor_copy(cyr, kb[:, :, 1:2].rearrange("g k o -> g (k o)"))
            wr = sb.tile([G, K], F32, tag="wr")
            nc.vector.tensor_copy(wr, kb[:, :, 2:3].rearrange("g k o -> g (k o)"))
            hr = sb.tile([G, K], F32, tag="hr")
            nc.vector.tensor_copy(hr, kb[:, :, 3:4].rearrange("g k o -> g (k o)"))

            kx1 = sb.tile([G, K], F32, tag="E0")
            kx2 = sb.tile([G, K], F32, tag="E1")
            ky1 = sb.tile([G, K], F32, tag="E2")
            ky2 = sb.tile([G, K], F32, tag="E3")
            t5 = sb.tile([G, K], F32, tag="E4")
            t6 = sb.tile([G, K], F32, tag="E5")
            t7 = sb.tile([G, K], F32, tag="E6")
            t8 = sb.tile([G, K], F32, tag="E7")
            nc.vector.scalar_tensor_tensor(
                kx1, in0=wr, scalar=-0.5, in1=cxr, op0=AL.mult, op1=AL.add)
            nc.vector.scalar_tensor_tensor(
                kx2, in0=wr, scalar=0.5, in1=cxr, op0=AL.mult, op1=AL.add)
            nc.vector.scalar_tensor_tensor(
                ky1, in0=hr, scalar=-0.5, in1=cyr, op0=AL.mult, op1=AL.add)
            nc.vector.scalar_tensor_tensor(
                ky2, in0=hr, scalar=0.5, in1=cyr, op0=AL.mult, op1=AL.add)

            nc.vector.tensor_sub(t7, kx2, kx1)               # abw
            nc.vector.tensor_sub(t8, ky2, ky1)               # abh
            nc.vector.tensor_mul(t7, t7, t8)                 # area_b

            nc.vector.tensor_scalar(t8, kx1, x1, None, op0=AL.max)   # ltx
            nc.vector.tensor_scalar(t5, kx2, x2, None, op0=AL.min)   # rbx
            nc.vector.tensor_sub(t5, t5, t8)                 # wx
            nc.vector.tensor_scalar(t8, ky1, y1, None, op0=AL.max)   # lty
            nc.vector.tensor_scalar(t6, ky2, y2, None, op0=AL.min)   # rby
            nc.vector.tensor_sub(t6, t6, t8)                 # wy
            nc.vector.tensor_scalar(t6, t6, 0.0, None, op0=AL.max)
            nc.vector.scalar_tensor_tensor(
                t8, in0=t5, scalar=0.0, in1=t6, op0=AL.max, op1=AL.mult)  # inter

            nc.vector.scalar_tensor_tensor(
                t5, in0=t7, scalar=area_a, in1=t8,
                op0=AL.add, op1=AL.subtract,
            )                                                # union
            nc.vector.reciprocal(t6, t5)                     # 1/union
            iou_r = sb.tile([G, K], F32, tag="iou_r")
            nc.vector.tensor_mul(iou_r, t8, t6)
            nc.sync.dma_start(out_ious.ap()[b], iou_r)

            # threshold = mean + std(ddof=1)
            msum = sb.tile([G, 1], F32, tag="msum")
            nc.vector.tensor_reduce(msum, iou_r, axis=AX.X, op=AL.add)
            mean = sb.tile([G, 1], F32, tag="mean")
            nc.vector.tensor_scalar_mul(mean, msum, 1.0 / K)
            nc.vector.tensor_scalar_sub(t5, iou_r, mean)     # cen
            nc.scalar.activation(t5, t5, ACT.Square)
            vsum = sb.tile([G, 1], F32, tag="vsum")
            nc.vector.tensor_reduce(vsum, t5, axis=AX.X, op=AL.add)
            var = sb.tile([G, 1], F32, tag="var")
            nc.vector.tensor_scalar_mul(
                var, vsum, float(np.float32(1.0) / np.float32(K - 1)))
            std = sb.tile([G, 1], F32, tag="std")
            nc.scalar.activation(std, var, ACT.Sqrt)
            thr = sb.tile([G, 1], F32, tag="thr")
            nc.scalar.activation(thr, std, ACT.Identity, bias=mean, scale=1.0)

            # mask = (iou >= thr) & inside
            nc.vector.tensor_scalar(t5, cxr, x1, None, op0=AL.is_ge)
            nc.vector.scalar_tensor_tensor(
                t6, in0=cxr, scalar=x2, in1=t5, op0=AL.is_le, op1=AL.mult)
            nc.vector.scalar_tensor_tensor(
                t5, in0=cyr, scalar=y1, in1=t6, op0=AL.is_ge, op1=AL.mult)
            nc.vector.scalar_tensor_tensor(
                t6, in0=cyr, scalar=y2, in1=t5, op0=AL.is_le, op1=AL.mult)
            nc.vector.scalar_tensor_tensor(
                t5, in0=iou_r, scalar=thr, in1=t6, op0=AL.is_ge, op1=AL.mult)

            msk8 = sb.tile([G, K], U8, tag="msk8")
            nc.vector.tensor_copy(msk8, t5)
            nc.sync.dma_start(out_mask.ap()[b], msk8)


_BUILT = None


def _shard_inputs(pred_boxes, gt_boxes):
    pred_boxes = np.asarray(pred_boxes, dtype=np.float32)
    gt_boxes = np.asarray(gt_boxes, dtype=np.float32)

    io_e = np.tile(np.arange(CW, dtype=np.uint16), NSEL)
    io_e = np.broadcast_to(io_e, (G, CAND)).copy()
    io_r = np.broadcast_to(
        np.arange(1, K + 1, dtype=np.int16), (G, K)).copy()

    in_maps = []
    for c in range(NCORES):
        pb = pred_boxes[c * BPC : (c + 1) * BPC]
        gb = gt_boxes[c * BPC : (c + 1) * BPC]
        ct = np.concatenate(
            [gb[:, :, :2].transpose(0, 2, 1), pb[:, :, :2].transpose(0, 2, 1)],
            axis=2,
        )
        # chunk-major interleave: [BPC, NCH, 4*CW] with planes cx, cy, w, h
        cc = pb.reshape(BPC, NCH, CW, 4).transpose(0, 1, 3, 2).reshape(
            BPC, NCH, 4 * CW)
        b64 = np.zeros((BPC, N, 64), dtype=np.float32)
        b64[:, :, 0:4] = pb
        in_maps.append({
            "ct_in": np.ascontiguousarray(ct),
            "pred_cc": np.ascontiguousarray(cc),
            "boxes64": b64,
            "gt_boxes": np.ascontiguousarray(gb),
            "io_e_in": io_e,
            "io_r_in": io_r,
        })
    return in_maps


def _assemble(results):
    ious = np.concatenate([results[c]["out_ious"] for c in range(NCORES)], axis=0)
    mask = np.concatenate([results[c]["out_mask"] for c in range(NCORES)], axis=0)
    kidx = np.concatenate([results[c]["out_kidx"] for c in range(NCORES)], axis=0)
    return (
        ious.astype(np.float32),
        mask.astype(bool),
        kidx.astype(np.int32),
    )


def kernel(pred_boxes, gt_boxes):
    global _BUILT
    from concourse.bass_utils import run_bass_kernel_spmd

    if _BUILT is None:
        _BUILT = build_program(NCORES)
    in_maps = _shard_inputs(pred_boxes, gt_boxes)
    res = run_bass_kernel_spmd(_BUILT, in_maps, core_ids=list(range(NCORES)))
    return _assemble(res.results)


# revision 11
# speedup vs baseline: 1.0800x; 1.0800x over previous
"""ATSS assignment kernel for Trainium2 (8 NeuronCores, data-parallel over batch).

Pipeline per core (2 images per core):
  1. One fused PE matmul per 512-chunk computes u = g.p - |p|^2/2 via a
     4-row contraction [gx, gy, -1/2, -1/2] x [px, py, px^2, py^2]
     (ranking-equivalent to negated squared distance).
  2. DVE tensor_reduce takes per-16-chunk maxima straight out of PSUM.
  3. A max8/max_index/match_replace cascade selects the top-72 chunks per
     GT row (any chunk holding a true top-64 pred ranks <= 64 by chunk-max).
  4. Chunk ids are transposed into the gpsimd wrapped-index layout with a
     few small DMAs and ALL 72*128 chunk blocks are fetched by a single
     dma_gather (SWDGE cost ~994ns + 0.34ns/desc instead of 994ns per
     block-column with per-slot indirect DMAs).
  5. Exact fp32 squared distances are recomputed for the 1152 candidates
     and a second cascade extracts the exact top-64 (jax top_k semantics).
  6. A rank map built by local_scatter compacts the original pred indices
     to rank order; a second dma_gather fetches the 64 selected boxes per
     GT row and the IoU / mean+std threshold / center-inside epilogue runs
     entirely in compact [G, 64] rank space.
"""

import sys

import numpy as np

if "/opt/trn_rl_repo" not in sys.path:
    sys.path.insert(0, "/opt/trn_rl_repo")

import concourse.bass as bass
import concourse.mybir as mybir
import concourse.tile as tile
from concourse import bacc

F32 = mybir.dt.float32
U8 = mybir.dt.uint8
U16 = mybir.dt.uint16
I16 = mybir.dt.int16
I32 = mybir.dt.int32
AL = mybir.AluOpType
ACT = mybir.ActivationFunctionType
AX = mybir.AxisListType

B, N, G, K = 16, 16384, 128, 64
NCORES = 8
BPC = B // NCORES          # batches (images) per core
CW = 16                    # chunk width for the prefilter
NCH = N // CW              # 1024 chunks per row
NSEL = 72                  # chunks kept per row (>= worst-case 65 + margin)
NIT1 = NSEL // 8           # cascade-1 iterations
CAND = NSEL * CW           # 1152 candidate preds per row
NIT2 = K // 8              # cascade-2 iterations
MMF = 512                  # matmul free-dim chunk (one PSUM bank)
NEG = -1e30


def _wrap_idx_dmas(nc, idxw, src16, ns):
    """Transpose [128, ns] per-partition ids into the gpsimd wrapped index
    layout idxw[16k+p, s*8+q] = src[q*16+p, s], replicated into all eight
    16-partition groups."""
    dst3 = idxw[0:16, :].rearrange("p (s q) -> p s q", q=8)
    for q in range(8):
        nc.sync.dma_start(dst3[:, :, q], src16[16 * q : 16 * (q + 1), :])
    for npart in (16, 32, 64):
        nc.sync.dma_start(idxw[npart : 2 * npart, :], idxw[0:npart, :])


def build_program(num_devices=NCORES, debug_taps=False, stop_after=99):
    nc = bacc.Bacc(
        "TRN2",
        debug=False,
        target_bir_lowering=False,
        num_devices=num_devices,
    )
    ct_in = nc.dram_tensor("ct_in", [BPC, 2, G + N], F32, kind="ExternalInput")
    pred_cc = nc.dram_tensor("pred_cc", [BPC, NCH, 4 * CW], F32, kind="ExternalInput")
    boxes64 = nc.dram_tensor("boxes64", [BPC, N, 64], F32, kind="ExternalInput")
    gt_boxes = nc.dram_tensor("gt_boxes", [BPC, G, 4], F32, kind="ExternalInput")
    io_e_in = nc.dram_tensor("io_e_in", [G, CAND], U16, kind="ExternalInput")
    io_r_in = nc.dram_tensor("io_r_in", [G, K], I16, kind="ExternalInput")
    out_ious = nc.dram_tensor("out_ious", [BPC, G, K], F32, kind="ExternalOutput")
    out_mask = nc.dram_tensor("out_mask", [BPC, G, K], U8, kind="ExternalOutput")
    out_kidx = nc.dram_tensor("out_kidx", [BPC, G, K], I32, kind="ExternalOutput")

    with tile.TileContext(nc) as tc:
        _emit(nc, tc, ct_in, pred_cc, boxes64, gt_boxes, io_e_in, io_r_in,
              out_ious, out_mask, out_kidx)
    nc.compile()
    return nc


def _emit(nc, tc, ct_in, pred_cc, boxes64, gt_boxes, io_e_in, io_r_in,
          out_ious, out_mask, out_kidx):
    with (
        tc.tile_pool(name="const", bufs=1) as cpool,
        tc.tile_pool(name="sb", bufs=2) as sb,
        tc.tile_pool(name="cw", bufs=1) as cw,
        tc.tile_pool(name="rhsp", bufs=2) as rhsp,
        tc.tile_pool(name="ps0", bufs=2, space="PSUM") as ps0,
        tc.tile_pool(name="ps1", bufs=2, space="PSUM") as ps1,
    ):
        # iota 0..15 repeated per chunk slot (j%16) and ranks 1..64
        io_e = cpool.tile([G, CAND], U16)
        nc.sync.dma_start(io_e, io_e_in.ap())
        io_r = cpool.tile([G, K], I16)
        nc.sync.dma_start(io_r, io_r_in.ap())

        per_b = [dict() for _ in range(BPC)]
        for b in range(BPC):
            # ---------------- GT prep ----------------
            gt_sb = sb.tile([G, 4], F32)
            nc.sync.dma_start(gt_sb, gt_boxes.ap()[b])
            gx = gt_sb[:, 0:1]
            gy = gt_sb[:, 1:2]

            ghw = sb.tile([G, 1], F32, tag="ghw")
            nc.vector.tensor_scalar_mul(ghw, gt_sb[:, 2:3], 0.5)
            ghh = sb.tile([G, 1], F32, tag="ghh")
            nc.vector.tensor_scalar_mul(ghh, gt_sb[:, 3:4], 0.5)
            x1 = sb.tile([G, 1], F32, tag="x1")
            nc.vector.tensor_sub(x1, gx, ghw)
            x2 = sb.tile([G, 1], F32, tag="x2")
            nc.vector.tensor_add(x2, gx, ghw)
            y1 = sb.tile([G, 1], F32, tag="y1")
            nc.vector.tensor_sub(y1, gy, ghh)
            y2 = sb.tile([G, 1], F32, tag="y2")
            nc.vector.tensor_add(y2, gy, ghh)
            aw = sb.tile([G, 1], F32, tag="aw")
            nc.vector.tensor_sub(aw, x2, x1)
            ah = sb.tile([G, 1], F32, tag="ah")
            nc.vector.tensor_sub(ah, y2, y1)
            area_a = sb.tile([G, 1], F32, tag="area_a")
            nc.vector.tensor_mul(area_a, aw, ah)

            # stage row c = pred_c (N); gt centers in a separate tile
            stage = rhsp.tile([2, N], F32)
            nc.sync.dma_start(stage.rearrange("c (s x) -> c s x", s=8),
                              ct_in.ap()[b][:, G:].rearrange(
                                  "c (s x) -> c s x", s=8))
            gtc = sb.tile([2, G], F32, tag="gtc")
            nc.sync.dma_start(gtc, ct_in.ap()[b][:, 0:G])
            negh = sb.tile([2, G], F32, tag="negh")
            nc.vector.memset(negh, -0.5)

            # ---------------- scores + chunk maxima ----------------
            m16 = sb.tile([G, NCH], F32)
            for j2 in range(N // (2 * MMF)):
                psp = ps0 if j2 % 2 == 0 else ps1
                mm = psp.tile([G, 2 * MMF], F32, tag="mm")
                for h in range(2):
                    j = 2 * j2 + h
                    sq2 = sb.tile([2, MMF], F32, tag="sq2")
                    nc.scalar.activation(
                        sq2, stage[:, j * MMF : (j + 1) * MMF],
                        ACT.Square)
                    nc.tensor.matmul(
                        mm[:, h * MMF : (h + 1) * MMF], lhsT=gtc,
                        rhs=stage[:, j * MMF : (j + 1) * MMF],
                        start=True, stop=False,
                    )
                    nc.tensor.matmul(
                        mm[:, h * MMF : (h + 1) * MMF], lhsT=negh, rhs=sq2,
                        start=False, stop=True,
                    )
                nc.vector.tensor_reduce(
                    m16[:, j2 * (2 * MMF // CW) : (j2 + 1) * (2 * MMF // CW)],
                    mm.rearrange("g (c w) -> g c w", w=CW),
                    axis=AX.X, op=AL.max,
                )

            # ---- cascade 1: top-NSEL chunks per GT row ----
            ci16 = sb.tile([G, NSEL], U16)
            for i in range(NIT1):
                v8 = sb.tile([G, 8], F32, tag="v8")
                nc.vector.max(v8, m16)
                nc.vector.max_index(ci16[:, i * 8 : (i + 1) * 8], v8, m16)
                if i < NIT1 - 1:
                    nc.vector.match_replace(m16, v8, m16, NEG)

            # ---- batched gather of all selected chunk blocks ----
            idxw = cw.tile([128, NSEL * 8], I16, tag="idxw")
            _wrap_idx_dmas(nc, idxw, ci16.bitcast(I16), NSEL)
            cand = cw.tile([G, NSEL, 4 * CW], F32, tag=f"cand{b}")
            nc.gpsimd.dma_gather(
                out_ap=cand, in_ap=pred_cc.ap()[b], idxs_ap=idxw,
                num_idxs=G * NSEL, num_idxs_reg=G * NSEL,
                elem_size=4 * CW, single_packet=False,
            )
            per_b[b] = dict(gt_sb=gt_sb, x1=x1, x2=x2, y1=y1, y2=y2,
                            area_a=area_a, ci16=ci16, cand=cand)

        for b in range(BPC):
            d = per_b[b]
            gt_sb, ci16, cand = d["gt_sb"], d["ci16"], d["cand"]
            gx = gt_sb[:, 0:1]
            gy = gt_sb[:, 1:2]
            cx_c = cand[:, :, 0 * CW : 1 * CW]
            cy_c = cand[:, :, 1 * CW : 2 * CW]

            # ---------------- exact negated d2 ----------------
            T1 = cw.tile([G, CAND], F32, tag="T1")
            T2 = cw.tile([G, CAND], F32, tag="T2")
            uc = T1

            nc.vector.tensor_scalar_sub(T1, cx_c, gx)        # dx
            nc.scalar.activation(T1, T1, ACT.Square)
            nc.vector.tensor_scalar_sub(T2, cy_c, gy)        # dy
            nc.scalar.activation(T2, T2, ACT.Square)
            nc.vector.scalar_tensor_tensor(
                uc, in0=T1, scalar=-1.0, in1=T2, op0=AL.mult, op1=AL.subtract
            )

            # ---------------- cascade 2: exact top-64 ----------------
            pos16 = sb.tile([G, K], U16)
            for i in range(NIT2):
                v8b = sb.tile([G, 8], F32, tag="v8b")
                nc.vector.max(v8b, uc)
                nc.vector.max_index(pos16[:, i * 8 : (i + 1) * 8], v8b, uc)
                if i < NIT2 - 1:
                    nc.vector.match_replace(uc, v8b, uc, NEG)

            # ---------------- rank map + original indices ----------------
            pos_i = sb.tile([G, K], I16, tag="pos_i")
            nc.vector.tensor_copy(pos_i, pos16)
            rkm = cw.tile([G, CAND], I16, tag="t_rkm")
            nc.gpsimd.local_scatter(rkm, io_r, pos_i, channels=G,
                                    num_elems=CAND, num_idxs=K)
            nc.vector.tensor_scalar(rkm, rkm, 1, None, op0=AL.subtract)

            # cid[g, j] = ci16[g, j//16]*16 + j%16  (global pred index)
            cid = cw.tile([G, CAND], U16, tag="t_cid")
            for e in range(CW):
                nc.scalar.activation(cid[:, e::CW], ci16, ACT.Copy, scale=16.0)
            nc.vector.tensor_tensor(cid, cid, io_e, op=AL.add)

            kidx16 = sb.tile([G, K], U16, tag="kidx16")
            nc.gpsimd.local_scatter(kidx16, cid, rkm, channels=G,
                                    num_elems=K, num_idxs=CAND)
            kidxi = sb.tile([G, K], I32, tag="kidxi")
            nc.vector.tensor_copy(kidxi, kidx16)
            nc.sync.dma_start(out_kidx.ap()[b], kidxi)

            # ---------------- gather the 64 ranked boxes ----------------
            idxw2 = cw.tile([128, K * 8], I16, tag="idxw2")
            _wrap_idx_dmas(nc, idxw2, kidx16.bitcast(I16), K)
            kbfull = cw.tile([G, NSEL, 4 * CW], F32, tag=f"cand{b}")
            kb = kbfull[:, 0:K, :]
            nc.gpsimd.dma_gather(
                out_ap=kb, in_ap=boxes64.ap()[b], idxs_ap=idxw2,
                num_idxs=G * K, num_idxs_reg=G * K,
                elem_size=64, single_packet=False,
            )
            d["kb"] = kb

        for b in range(BPC):
            d = per_b[b]
            x1, x2, y1, y2 = d["x1"], d["x2"], d["y1"], d["y2"]
            area_a, kb = d["area_a"], d["kb"]

            # ---------------- rank-space epilogue [G, K] ----------------
            cxr = sb.tile([G, K], F32, tag="cxr")
            nc.vector.tensor_copy(cxr, kb[:, :, 0:1].rearrange("g k o -> g (k o)"))
            cyr = sb.tile([G, K], F32, tag="cyr")
            nc.vector.tensor_copy(cyr, kb[:, :, 1:2].rearrange("g k o -> g (k o)"))
            wr = sb.tile([G, K], F32, tag="wr")
            nc.vector.tensor_copy(wr, kb[:, :, 2:3].rearrange("g k o -> g (k o)"))
            hr = sb.tile([G, K], F32, tag="hr")
            nc.vector.tensor_copy(hr, kb[:, :, 3:4].rearrange("g k o -> g (k o)"))

            kx1 = sb.tile([G, K], F32, tag="E0")
            kx2 = sb.tile([G, K], F32, tag="E1")
            ky1 = sb.tile([G, K], F32, tag="E2")
            ky2 = sb.tile([G, K], F32, tag="E3")
            t5 = sb.tile([G, K], F32, tag="E4")
            t6 = sb.tile([G, K], F32, tag="E5")
            t7 = sb.tile([G, K], F32, tag="E6")
            t8 = sb.tile([G, K], F32, tag="E7")
            nc.vector.scalar_tensor_tensor(
                kx1, in0=wr, scalar=-0.5, in1=cxr, op0=AL.mult, op1=AL.add)
            nc.vector.scalar_tensor_tensor(
                kx2, in0=wr, scalar=0.5, in1=cxr, op0=AL.mult, op1=AL.add)
            nc.vector.scalar_tensor_tensor(
                ky1, in0=hr, scalar=-0.5, in1=cyr, op0=AL.mult, op1=AL.add)
            nc.vector.scalar_tensor_tensor(
                ky2, in0=hr, scalar=0.5, in1=cyr, op0=AL.mult, op1=AL.add)

            nc.vector.tensor_sub(t7, kx2, kx1)               # abw
            nc.vector.tensor_sub(t8, ky2, ky1)               # abh
            nc.vector.tensor_mul(t7, t7, t8)                 # area_b

            nc.vector.tensor_scalar(t8, kx1, x1, None, op0=AL.max)   # ltx
            nc.vector.tensor_scalar(t5, kx2, x2, None, op0=AL.min)   # rbx
            nc.vector.tensor_sub(t5, t5, t8)                 # wx
            nc.vector.tensor_scalar(t8, ky1, y1, None, op0=AL.max)   # lty
            nc.vector.tensor_scalar(t6, ky2, y2, None, op0=AL.min)   # rby
            nc.vector.tensor_sub(t6, t6, t8)                 # wy
            nc.vector.tensor_scalar(t6, t6, 0.0, None, op0=AL.max)
            nc.vector.scalar_tensor_tensor(
                t8, in0=t5, scalar=0.0, in1=t6, op0=AL.max, op1=AL.mult)  # inter

            nc.vector.scalar_tensor_tensor(
                t5, in0=t7, scalar=area_a, in1=t8,
                op0=AL.add, op1=AL.subtract,
            )                                                # union
            nc.vector.reciprocal(t6, t5)                     # 1/union
            iou_r = sb.tile([G, K], F32, tag="iou_r")
            nc.vector.tensor_mul(iou_r, t8, t6)
            nc.sync.dma_start(out_ious.ap()[b], iou_r)

            # threshold = mean + std(ddof=1)
            msum = sb.tile([G, 1], F32, tag="msum")
            nc.vector.tensor_reduce(msum, iou_r, axis=AX.X, op=AL.add)
            mean = sb.tile([G, 1], F32, tag="mean")
            nc.vector.tensor_scalar_mul(mean, msum, 1.0 / K)
            nc.vector.tensor_scalar_sub(t5, iou_r, mean)     # cen
            nc.scalar.activation(t5, t5, ACT.Square)
            vsum = sb.tile([G, 1], F32, tag="vsum")
            nc.vector.tensor_reduce(vsum, t5, axis=AX.X, op=AL.add)
            var = sb.tile([G, 1], F32, tag="var")
            nc.vector.tensor_scalar_mul(
                var, vsum, float(np.float32(1.0) / np.float32(K - 1)))
            std = sb.tile([G, 1], F32, tag="std")
            nc.scalar.activation(std, var, ACT.Sqrt)
            thr = sb.tile([G, 1], F32, tag="thr")
            nc.scalar.activation(thr, std, ACT.Identity, bias=mean, scale=1.0)

            # mask = (iou >= thr) & inside
            nc.vector.tensor_scalar(t5, cxr, x1, None, op0=AL.is_ge)
            nc.vector.scalar_tensor_tensor(
                t6, in0=cxr, scalar=x2, in1=t5, op0=AL.is_le, op1=AL.mult)
            nc.vector.scalar_tensor_tensor(
                t5, in0=cyr, scalar=y1, in1=t6, op0=AL.is_ge, op1=AL.mult)
            nc.vector.scalar_tensor_tensor(
                t6, in0=cyr, scalar=y2, in1=t5, op0=AL.is_le, op1=AL.mult)
            nc.vector.scalar_tensor_tensor(
                t5, in0=iou_r, scalar=thr, in1=t6, op0=AL.is_ge, op1=AL.mult)

            msk8 = sb.tile([G, K], U8, tag="msk8")
            nc.vector.tensor_copy(msk8, t5)
            nc.sync.dma_start(out_mask.ap()[b], msk8)


_BUILT = None


def _shard_inputs(pred_boxes, gt_boxes):
    pred_boxes = np.asarray(pred_boxes, dtype=np.float32)
    gt_boxes = np.asarray(gt_boxes, dtype=np.float32)

    io_e = np.tile(np.arange(CW, dtype=np.uint16), NSEL)
    io_e = np.broadcast_to(io_e, (G, CAND)).copy()
    io_r = np.broadcast_to(
        np.arange(1, K + 1, dtype=np.int16), (G, K)).copy()

    in_maps = []
    for c in range(NCORES):
        pb = pred_boxes[c * BPC : (c + 1) * BPC]
        gb = gt_boxes[c * BPC : (c + 1) * BPC]
        ct = np.concatenate(
            [gb[:, :, :2].transpose(0, 2, 1), pb[:, :, :2].transpose(0, 2, 1)],
            axis=2,
        )
        # chunk-major interleave: [BPC, NCH, 4*CW] with planes cx, cy, w, h
        cc = pb.reshape(BPC, NCH, CW, 4).transpose(0, 1, 3, 2).reshape(
            BPC, NCH, 4 * CW)
        b64 = np.zeros((BPC, N, 64), dtype=np.float32)
        b64[:, :, 0:4] = pb
        in_maps.append({
            "ct_in": np.ascontiguousarray(ct),
            "pred_cc": np.ascontiguousarray(cc),
            "boxes64": b64,
            "gt_boxes": np.ascontiguousarray(gb),
            "io_e_in": io_e,
            "io_r_in": io_r,
        })
    return in_maps


def _assemble(results):
    ious = np.concatenate([results[c]["out_ious"] for c in range(NCORES)], axis=0)
    mask = np.concatenate([results[c]["out_mask"] for c in range(NCORES)], axis=0)
    kidx = np.concatenate([results[c]["out_kidx"] for c in range(NCORES)], axis=0)
    return (
        ious.astype(np.float32),
        mask.astype(bool),
        kidx.astype(np.int32),
    )


def kernel(pred_boxes, gt_boxes):
    global _BUILT
    from concourse.bass_utils import run_bass_kernel_spmd

    if _BUILT is None:
        _BUILT = build_program(NCORES)
    in_maps = _shard_inputs(pred_boxes, gt_boxes)
    res = run_bass_kernel_spmd(_BUILT, in_maps, core_ids=list(range(NCORES)))
    return _assemble(res.results)


# revision 12
# speedup vs baseline: 1.0830x; 1.0028x over previous
"""ATSS assignment kernel for Trainium2 (8 NeuronCores, data-parallel over batch).

Pipeline per core (2 images per core):
  1. One fused PE matmul per 512-chunk computes u = g.p - |p|^2/2 via a
     4-row contraction [gx, gy, -1/2, -1/2] x [px, py, px^2, py^2]
     (ranking-equivalent to negated squared distance).
  2. DVE tensor_reduce takes per-16-chunk maxima straight out of PSUM.
  3. A max8/max_index/match_replace cascade selects the top-72 chunks per
     GT row (any chunk holding a true top-64 pred ranks <= 64 by chunk-max).
  4. Chunk ids are transposed into the gpsimd wrapped-index layout with a
     few small DMAs and ALL 72*128 chunk blocks are fetched by a single
     dma_gather (SWDGE cost ~994ns + 0.34ns/desc instead of 994ns per
     block-column with per-slot indirect DMAs).
  5. Exact fp32 squared distances are recomputed for the 1152 candidates
     and a second cascade extracts the exact top-64 (jax top_k semantics).
  6. A rank map built by local_scatter compacts the original pred indices
     to rank order; a second dma_gather fetches the 64 selected boxes per
     GT row and the IoU / mean+std threshold / center-inside epilogue runs
     entirely in compact [G, 64] rank space.
"""

import sys

import numpy as np

if "/opt/trn_rl_repo" not in sys.path:
    sys.path.insert(0, "/opt/trn_rl_repo")

import concourse.bass as bass
import concourse.mybir as mybir
import concourse.tile as tile
from concourse import bacc

F32 = mybir.dt.float32
U8 = mybir.dt.uint8
U16 = mybir.dt.uint16
I16 = mybir.dt.int16
I32 = mybir.dt.int32
AL = mybir.AluOpType
ACT = mybir.ActivationFunctionType
AX = mybir.AxisListType

B, N, G, K = 16, 16384, 128, 64
NCORES = 8
BPC = B // NCORES          # batches (images) per core
CW = 16                    # chunk width for the prefilter
NCH = N // CW              # 1024 chunks per row
NSEL = 72                  # chunks kept per row (>= worst-case 65 + margin)
NIT1 = NSEL // 8           # cascade-1 iterations
CAND = NSEL * CW           # 1152 candidate preds per row
NIT2 = K // 8              # cascade-2 iterations
MMF = 512                  # matmul free-dim chunk (one PSUM bank)
NEG = -1e30


def _wrap_idx_dmas(nc, idxw, src16, ns):
    """Transpose [128, ns] per-partition ids into the gpsimd wrapped index
    layout idxw[16k+p, s*8+q] = src[q*16+p, s], replicated into all eight
    16-partition groups."""
    dst3 = idxw[0:16, :].rearrange("p (s q) -> p s q", q=8)
    for q in range(8):
        nc.sync.dma_start(dst3[:, :, q], src16[16 * q : 16 * (q + 1), :])
    for npart in (16, 32, 64):
        nc.sync.dma_start(idxw[npart : 2 * npart, :], idxw[0:npart, :])


def build_program(num_devices=NCORES, debug_taps=False, stop_after=99):
    nc = bacc.Bacc(
        "TRN2",
        debug=False,
        target_bir_lowering=False,
        num_devices=num_devices,
    )
    ct_in = nc.dram_tensor("ct_in", [BPC, 2, G + N], F32, kind="ExternalInput")
    pred_cc = nc.dram_tensor("pred_cc", [BPC, NCH, 4 * CW], F32, kind="ExternalInput")
    boxes64 = nc.dram_tensor("boxes64", [BPC, N, 64], F32, kind="ExternalInput")
    gt_boxes = nc.dram_tensor("gt_boxes", [BPC, G, 4], F32, kind="ExternalInput")
    io_e_in = nc.dram_tensor("io_e_in", [G, CAND], U16, kind="ExternalInput")
    io_r_in = nc.dram_tensor("io_r_in", [G, K], I16, kind="ExternalInput")
    out_ious = nc.dram_tensor("out_ious", [BPC, G, K], F32, kind="ExternalOutput")
    out_mask = nc.dram_tensor("out_mask", [BPC, G, K], U8, kind="ExternalOutput")
    out_kidx = nc.dram_tensor("out_kidx", [BPC, G, K], I32, kind="ExternalOutput")

    with tile.TileContext(nc) as tc:
        _emit(nc, tc, ct_in, pred_cc, boxes64, gt_boxes, io_e_in, io_r_in,
              out_ious, out_mask, out_kidx)
    nc.compile()
    return nc


def _emit(nc, tc, ct_in, pred_cc, boxes64, gt_boxes, io_e_in, io_r_in,
          out_ious, out_mask, out_kidx):
    with (
        tc.tile_pool(name="const", bufs=1) as cpool,
        tc.tile_pool(name="sb", bufs=2) as sb,
        tc.tile_pool(name="cw", bufs=1) as cw,
        tc.tile_pool(name="rhsp", bufs=2) as rhsp,
        tc.tile_pool(name="ps0", bufs=2, space="PSUM") as ps0,
        tc.tile_pool(name="ps1", bufs=2, space="PSUM") as ps1,
    ):
        # iota 0..15 repeated per chunk slot (j%16) and ranks 1..64
        io_e = cpool.tile([G, CAND], U16)
        nc.sync.dma_start(io_e, io_e_in.ap())
        io_r = cpool.tile([G, K], I16)
        nc.sync.dma_start(io_r, io_r_in.ap())

        per_b = [dict() for _ in range(BPC)]
        for b in range(BPC):
            # ---------------- GT prep ----------------
            gt_sb = sb.tile([G, 4], F32)
            nc.sync.dma_start(gt_sb, gt_boxes.ap()[b])
            gx = gt_sb[:, 0:1]
            gy = gt_sb[:, 1:2]

            ghw = sb.tile([G, 1], F32, tag="ghw")
            nc.vector.tensor_scalar_mul(ghw, gt_sb[:, 2:3], 0.5)
            ghh = sb.tile([G, 1], F32, tag="ghh")
            nc.vector.tensor_scalar_mul(ghh, gt_sb[:, 3:4], 0.5)
            x1 = sb.tile([G, 1], F32, tag="x1")
            nc.vector.tensor_sub(x1, gx, ghw)
            x2 = sb.tile([G, 1], F32, tag="x2")
            nc.vector.tensor_add(x2, gx, ghw)
            y1 = sb.tile([G, 1], F32, tag="y1")
            nc.vector.tensor_sub(y1, gy, ghh)
            y2 = sb.tile([G, 1], F32, tag="y2")
            nc.vector.tensor_add(y2, gy, ghh)
            aw = sb.tile([G, 1], F32, tag="aw")
            nc.vector.tensor_sub(aw, x2, x1)
            ah = sb.tile([G, 1], F32, tag="ah")
            nc.vector.tensor_sub(ah, y2, y1)
            area_a = sb.tile([G, 1], F32, tag="area_a")
            nc.vector.tensor_mul(area_a, aw, ah)

            # stage row c = pred_c (N); gt centers in a separate tile
            stage = rhsp.tile([2, N], F32)
            nc.sync.dma_start(stage.rearrange("c (s x) -> c s x", s=8),
                              ct_in.ap()[b][:, G:].rearrange(
                                  "c (s x) -> c s x", s=8))
            gtc = sb.tile([2, G], F32, tag="gtc")
            nc.sync.dma_start(gtc, ct_in.ap()[b][:, 0:G])
            negh = sb.tile([2, G], F32, tag="negh")
            nc.vector.memset(negh, -0.5)

            # ---------------- scores + chunk maxima ----------------
            m16 = sb.tile([G, NCH], F32)
            for j2 in range(N // (2 * MMF)):
                psp = ps0 if j2 % 2 == 0 else ps1
                mm = psp.tile([G, 2 * MMF], F32, tag="mm")
                for h in range(2):
                    j = 2 * j2 + h
                    sq2 = sb.tile([2, MMF], F32, tag="sq2")
                    nc.scalar.activation(
                        sq2, stage[:, j * MMF : (j + 1) * MMF],
                        ACT.Square)
                    nc.tensor.matmul(
                        mm[:, h * MMF : (h + 1) * MMF], lhsT=gtc,
                        rhs=stage[:, j * MMF : (j + 1) * MMF],
                        start=True, stop=False,
                    )
                    nc.tensor.matmul(
                        mm[:, h * MMF : (h + 1) * MMF], lhsT=negh, rhs=sq2,
                        start=False, stop=True,
                    )
                nc.vector.tensor_reduce(
                    m16[:, j2 * (2 * MMF // CW) : (j2 + 1) * (2 * MMF // CW)],
                    mm.rearrange("g (c w) -> g c w", w=CW),
                    axis=AX.X, op=AL.max,
                )

            # ---- cascade 1: top-NSEL chunks per GT row ----
            # each iteration's 8 chunk ids are transposed into the wrapped
            # index layout immediately, so the gather can start right after
            # the last iteration
            ci16 = sb.tile([G, NSEL], U16)
            idxw = cw.tile([128, NSEL * 8], I16, tag="idxw")
            ci_i16 = ci16.bitcast(I16)
            dst4 = idxw.rearrange("p (s q) -> p s q", q=8)
            for i in range(NIT1):
                v8 = sb.tile([G, 8], F32, tag="v8")
                nc.vector.max(v8, m16)
                nc.vector.max_index(ci16[:, i * 8 : (i + 1) * 8], v8, m16)
                if i < NIT1 - 1:
                    nc.vector.match_replace(m16, v8, m16, NEG)
                for q in range(8):
                    for k in range(8):
                        pass
                sl = slice(i * 8, (i + 1) * 8)
                for q in range(8):
                    nc.sync.dma_start(
                        dst4[0:16, sl, q],
                        ci_i16[16 * q : 16 * (q + 1), sl])
            for npart in (16, 32, 64):
                nc.sync.dma_start(idxw[npart : 2 * npart, :], idxw[0:npart, :])
            cand = cw.tile([G, NSEL, 4 * CW], F32, tag=f"cand{b}")
            nc.gpsimd.dma_gather(
                out_ap=cand, in_ap=pred_cc.ap()[b], idxs_ap=idxw,
                num_idxs=G * NSEL, num_idxs_reg=G * NSEL,
                elem_size=4 * CW, single_packet=False,
            )
            per_b[b] = dict(gt_sb=gt_sb, x1=x1, x2=x2, y1=y1, y2=y2,
                            area_a=area_a, ci16=ci16, cand=cand)

        for b in range(BPC):
            d = per_b[b]
            gt_sb, ci16, cand = d["gt_sb"], d["ci16"], d["cand"]
            gx = gt_sb[:, 0:1]
            gy = gt_sb[:, 1:2]
            cx_c = cand[:, :, 0 * CW : 1 * CW]
            cy_c = cand[:, :, 1 * CW : 2 * CW]

            # ---------------- exact negated d2 ----------------
            T1 = cw.tile([G, CAND], F32, tag="T1")
            T2 = cw.tile([G, CAND], F32, tag="T2")
            uc = T1

            nc.vector.tensor_scalar_sub(T1, cx_c, gx)        # dx
            nc.scalar.activation(T1, T1, ACT.Square)
            nc.vector.tensor_scalar_sub(T2, cy_c, gy)        # dy
            nc.scalar.activation(T2, T2, ACT.Square)
            nc.vector.scalar_tensor_tensor(
                uc, in0=T1, scalar=-1.0, in1=T2, op0=AL.mult, op1=AL.subtract
            )

            # ---------------- cascade 2: exact top-64 ----------------
            pos16 = sb.tile([G, K], U16)
            for i in range(NIT2):
                v8b = sb.tile([G, 8], F32, tag="v8b")
                nc.vector.max(v8b, uc)
                nc.vector.max_index(pos16[:, i * 8 : (i + 1) * 8], v8b, uc)
                if i < NIT2 - 1:
                    nc.vector.match_replace(uc, v8b, uc, NEG)

            # ---------------- rank map + original indices ----------------
            pos_i = sb.tile([G, K], I16, tag="pos_i")
            nc.vector.tensor_copy(pos_i, pos16)
            rkm = cw.tile([G, CAND], I16, tag="t_rkm")
            nc.gpsimd.local_scatter(rkm, io_r, pos_i, channels=G,
                                    num_elems=CAND, num_idxs=K)
            nc.vector.tensor_scalar(rkm, rkm, 1, None, op0=AL.subtract)

            # cid[g, j] = ci16[g, j//16]*16 + j%16  (global pred index)
            cid = cw.tile([G, CAND], U16, tag="t_cid")
            for e in range(CW):
                nc.scalar.activation(cid[:, e::CW], ci16, ACT.Copy, scale=16.0)
            nc.vector.tensor_tensor(cid, cid, io_e, op=AL.add)

            kidx16 = sb.tile([G, K], U16, tag="kidx16")
            nc.gpsimd.local_scatter(kidx16, cid, rkm, channels=G,
                                    num_elems=K, num_idxs=CAND)
            kidxi = sb.tile([G, K], I32, tag="kidxi")
            nc.vector.tensor_copy(kidxi, kidx16)
            nc.sync.dma_start(out_kidx.ap()[b], kidxi)

            # ---------------- gather the 64 ranked boxes ----------------
            idxw2 = cw.tile([128, K * 8], I16, tag="idxw2")
            _wrap_idx_dmas(nc, idxw2, kidx16.bitcast(I16), K)
            kbfull = cw.tile([G, NSEL, 4 * CW], F32, tag=f"cand{b}")
            kb = kbfull[:, 0:K, :]
            nc.gpsimd.dma_gather(
                out_ap=kb, in_ap=boxes64.ap()[b], idxs_ap=idxw2,
                num_idxs=G * K, num_idxs_reg=G * K,
                elem_size=64, single_packet=False,
            )
            d["kb"] = kb

        for b in range(BPC):
            d = per_b[b]
            x1, x2, y1, y2 = d["x1"], d["x2"], d["y1"], d["y2"]
            area_a, kb = d["area_a"], d["kb"]

            # ---------------- rank-space epilogue [G, K] ----------------
            cxr = sb.tile([G, K], F32, tag="cxr")
            nc.vector.tensor_copy(cxr, kb[:, :, 0:1].rearrange("g k o -> g (k o)"))
            cyr = sb.tile([G, K], F32, tag="cyr")
            nc.vector.tensor_copy(cyr, kb[:, :, 1:2].rearrange("g k o -> g (k o)"))
            wr = sb.tile([G, K], F32, tag="wr")
            nc.vector.tensor_copy(wr, kb[:, :, 2:3].rearrange("g k o -> g (k o)"))
            hr = sb.tile([G, K], F32, tag="hr")
            nc.vector.tensor_copy(hr, kb[:, :, 3:4].rearrange("g k o -> g (k o)"))

            kx1 = sb.tile([G, K], F32, tag="E0")
            kx2 = sb.tile([G, K], F32, tag="E1")
            ky1 = sb.tile([G, K], F32, tag="E2")
            ky2 = sb.tile([G, K], F32, tag="E3")
            t5 = sb.tile([G, K], F32, tag="E4")
            t6 = sb.tile([G, K], F32, tag="E5")
            t7 = sb.tile([G, K], F32, tag="E6")
            t8 = sb.tile([G, K], F32, tag="E7")
            nc.vector.scalar_tensor_tensor(
                kx1, in0=wr, scalar=-0.5, in1=cxr, op0=AL.mult, op1=AL.add)
            nc.vector.scalar_tensor_tensor(
                kx2, in0=wr, scalar=0.5, in1=cxr, op0=AL.mult, op1=AL.add)
            nc.vector.scalar_tensor_tensor(
                ky1, in0=hr, scalar=-0.5, in1=cyr, op0=AL.mult, op1=AL.add)
            nc.vector.scalar_tensor_tensor(
                ky2, in0=hr, scalar=0.5, in1=cyr, op0=AL.mult, op1=AL.add)

            nc.vector.tensor_sub(t7, kx2, kx1)               # abw
            nc.vector.tensor_sub(t8, ky2, ky1)               # abh
            nc.vector.tensor_mul(t7, t7, t8)                 # area_b

            nc.vector.tensor_scalar(t8, kx1, x1, None, op0=AL.max)   # ltx
            nc.vector.tensor_scalar(t5, kx2, x2, None, op0=AL.min)   # rbx
            nc.vector.tensor_sub(t5, t5, t8)                 # wx
            nc.vector.tensor_scalar(t8, ky1, y1, None, op0=AL.max)   # lty
            nc.vector.tensor_scalar(t6, ky2, y2, None, op0=AL.min)   # rby
            nc.vector.tensor_sub(t6, t6, t8)                 # wy
            nc.vector.tensor_scalar(t6, t6, 0.0, None, op0=AL.max)
            nc.vector.scalar_tensor_tensor(
                t8, in0=t5, scalar=0.0, in1=t6, op0=AL.max, op1=AL.mult)  # inter

            nc.vector.scalar_tensor_tensor(
                t5, in0=t7, scalar=area_a, in1=t8,
                op0=AL.add, op1=AL.subtract,
            )                                                # union
            nc.vector.reciprocal(t6, t5)                     # 1/union
            iou_r = sb.tile([G, K], F32, tag="iou_r")
            nc.vector.tensor_mul(iou_r, t8, t6)
            nc.sync.dma_start(out_ious.ap()[b], iou_r)

            # threshold = mean + std(ddof=1)
            msum = sb.tile([G, 1], F32, tag="msum")
            nc.vector.tensor_reduce(msum, iou_r, axis=AX.X, op=AL.add)
            mean = sb.tile([G, 1], F32, tag="mean")
            nc.vector.tensor_scalar_mul(mean, msum, 1.0 / K)
            nc.vector.tensor_scalar_sub(t5, iou_r, mean)     # cen
            nc.scalar.activation(t5, t5, ACT.Square)
            vsum = sb.tile([G, 1], F32, tag="vsum")
            nc.vector.tensor_reduce(vsum, t5, axis=AX.X, op=AL.add)
            var = sb.tile([G, 1], F32, tag="var")
            nc.vector.tensor_scalar_mul(
                var, vsum, float(np.float32(1.0) / np.float32(K - 1)))
            std = sb.tile([G, 1], F32, tag="std")
            nc.scalar.activation(std, var, ACT.Sqrt)
            thr = sb.tile([G, 1], F32, tag="thr")
            nc.scalar.activation(thr, std, ACT.Identity, bias=mean, scale=1.0)

            # mask = (iou >= thr) & inside
            nc.vector.tensor_scalar(t5, cxr, x1, None, op0=AL.is_ge)
            nc.vector.scalar_tensor_tensor(
                t6, in0=cxr, scalar=x2, in1=t5, op0=AL.is_le, op1=AL.mult)
            nc.vector.scalar_tensor_tensor(
                t5, in0=cyr, scalar=y1, in1=t6, op0=AL.is_ge, op1=AL.mult)
            nc.vector.scalar_tensor_tensor(
                t6, in0=cyr, scalar=y2, in1=t5, op0=AL.is_le, op1=AL.mult)
            nc.vector.scalar_tensor_tensor(
                t5, in0=iou_r, scalar=thr, in1=t6, op0=AL.is_ge, op1=AL.mult)

            msk8 = sb.tile([G, K], U8, tag="msk8")
            nc.vector.tensor_copy(msk8, t5)
            nc.sync.dma_start(out_mask.ap()[b], msk8)


_BUILT = None


def _shard_inputs(pred_boxes, gt_boxes):
    pred_boxes = np.asarray(pred_boxes, dtype=np.float32)
    gt_boxes = np.asarray(gt_boxes, dtype=np.float32)

    io_e = np.tile(np.arange(CW, dtype=np.uint16), NSEL)
    io_e = np.broadcast_to(io_e, (G, CAND)).copy()
    io_r = np.broadcast_to(
        np.arange(1, K + 1, dtype=np.int16), (G, K)).copy()

    in_maps = []
    for c in range(NCORES):
        pb = pred_boxes[c * BPC : (c + 1) * BPC]
        gb = gt_boxes[c * BPC : (c + 1) * BPC]
        ct = np.concatenate(
            [gb[:, :, :2].transpose(0, 2, 1), pb[:, :, :2].transpose(0, 2, 1)],
            axis=2,
        )
        # chunk-major interleave: [BPC, NCH, 4*CW] with planes cx, cy, w, h
        cc = pb.reshape(BPC, NCH, CW, 4).transpose(0, 1, 3, 2).reshape(
            BPC, NCH, 4 * CW)
        b64 = np.zeros((BPC, N, 64), dtype=np.float32)
        b64[:, :, 0:4] = pb
        in_maps.append({
            "ct_in": np.ascontiguousarray(ct),
            "pred_cc": np.ascontiguousarray(cc),
            "boxes64": b64,
            "gt_boxes": np.ascontiguousarray(gb),
            "io_e_in": io_e,
            "io_r_in": io_r,
        })
    return in_maps


def _assemble(results):
    ious = np.concatenate([results[c]["out_ious"] for c in range(NCORES)], axis=0)
    mask = np.concatenate([results[c]["out_mask"] for c in range(NCORES)], axis=0)
    kidx = np.concatenate([results[c]["out_kidx"] for c in range(NCORES)], axis=0)
    return (
        ious.astype(np.float32),
        mask.astype(bool),
        kidx.astype(np.int32),
    )


def kernel(pred_boxes, gt_boxes):
    global _BUILT
    from concourse.bass_utils import run_bass_kernel_spmd

    if _BUILT is None:
        _BUILT = build_program(NCORES)
    in_maps = _shard_inputs(pred_boxes, gt_boxes)
    res = run_bass_kernel_spmd(_BUILT, in_maps, core_ids=list(range(NCORES)))
    return _assemble(res.results)
